# revision 53
# baseline (speedup 1.0000x reference)
# Trainium2 Bass kernel for the 5-branch channel-attention module.
#
# Layout of the computation per batch sample n:
#   avg/max pool of x[n, :, :, TORSO] over (T, torso joints) -> p[c, {avg,max}]
#   h    = relu(W1 @ p + b1)                    (5 branches, HID=16)
#   g    = sigmoid(W2 @ (h_avg + h_max) + 2*b2) (per branch, per channel)
#   out[n, c, t, j] = x[n, c, t, perm[j]] * g[group(j), c]
#
# Sharding: pure data parallel, batch N=64 split over 8 cores (8 samples
# each); the tiny MLP weights are replicated.
#
# Performance strategy (target_regime=memory):
#  * x and out are streamed as fp16, halving HBM traffic to ~6.8 MiB each
#    way per core -> ~31 us DMA floor at 435 GB/s (harness gate is 2e-2;
#    fp16 end-to-end adds ~3e-3 worst-case elementwise error).
#  * The host permutes x's joint columns into OUTPUT-group order and pads
#    V 25->26 (zero pad col between torso and the rest): the torso pool is
#    one contiguous run, every gating multiply is a 4B-aligned even-extent
#    contiguous fp16 run, and the output needs no permutation (the
#    reference output is already group-concatenated).
#  * avg-pool runs on the TENSOR engine: since W1 and the average are both
#    linear, compute y = W1^T @ x_torso (PE matmul, accumulated over both
#    channel chunks) and reduce y's 320 columns on DVE - no ACT pooling.
#  * max-pool runs on gpsimd (reduce_max), off the DVE/ACT critical path.
#  * All weight reshuffling happens on the host (one fp32 + one fp16
#    constant block, two contiguous DMAs).
#  * The sync engine issues ALL DMAs: the 16 input loads up front
#    (prefetch, SBUF holds the full shard), output stores behind them.
#  * Gate pipeline is batched over sample pairs; gating multiplies are
#    split DVE(3 runs)/ACT(1)/gpsimd(1) to balance engine load.

import numpy as np
from contextlib import ExitStack

import concourse.bass as bass
import concourse.bacc as bacc
import concourse.tile as tile
from concourse import mybir
from concourse.bass_utils import run_bass_kernel_spmd

N, C, T, V = 64, 256, 64, 25
VD = 26                     # device joint dim (pad col 5 = 0)
HID = 16
NF = 5
NCORES = 8
NLOC = N // NCORES          # samples per core
NCH = C // 128              # channel chunks of 128 partitions
NPAIR = NLOC // 2
POOLSZ = T * 5              # elements pooled per channel (T x 5 torso joints)
K2 = 96 + NF                # layer-2 contraction rows (80 W2 + pad + 5 bias)

F32 = mybir.dt.float32
F16 = mybir.dt.float16

TORSO = [0, 1, 2, 3, 20]
LEFT_HAND = [8, 9, 10, 11, 23, 24]
LEFT_LEG = [16, 17, 18, 19]
RIGHT_HAND = [4, 5, 6, 7, 21, 22]
RIGHT_LEG = [12, 13, 14, 15]
DEV_SRC = TORSO + [0] + LEFT_HAND + LEFT_LEG + RIGHT_HAND + RIGHT_LEG
PAD_COL = 5
DEV_SEL = [j for j in range(VD) if j != PAD_COL]
# per-device-column group index (pad col rides with group 0: 0 * gate = 0)
COL_GRP = [0] * 6 + [1] * 6 + [2] * 4 + [3] * 6 + [4] * 4
# gating: cols [0:TT_COLS) in ONE fp16 tensor_tensor per chunk-sample on
# DVE (gate row broadcast over t); the tail run goes to ACT.
TT_COLS = 22
ACT_RUN = (4, 22, 4)      # (group, start_col, n_cols)

# fp32 constant block cw [128, CW_COLS]:
#   [0:256)   w2k      (rows 0:80 = W2s[f,c,h] at row f*16+h; rows 96:101 = 2*b2s)
#   [256:282) bdmask26 ([101, 26] block-diag mask expanded to device cols)
#   [282:283) negb1    ([80, 1])
#   [283:284) b1x2     ([80, 1])
CW_COLS = 284

_CACHE: dict = {}


def _pack_consts(W1s, b1s, W2s, b2s):
    cw = np.zeros((128, CW_COLS), dtype=np.float32)
    w2k = np.zeros((K2, C), dtype=np.float32)
    w2k[0:NF * HID] = W2s.transpose(0, 2, 1).reshape(NF * HID, C)
    w2k[96:K2] = 2.0 * b2s
    cw[0:K2, 0:256] = w2k
    bd = np.zeros((K2, NF), dtype=np.float32)
    for f in range(NF):
        bd[f * HID:(f + 1) * HID, f] = 1.0
        bd[96 + f, f] = 1.0
    cw[0:K2, 256:282] = bd[:, COL_GRP]
    b1f = b1s.reshape(NF * HID)
    cw[0:NF * HID, 282] = -b1f
    cw[0:NF * HID, 283] = 2.0 * b1f
    # fp16 W1 stationary: w1h[c', ch*80 + f*16+h] = W1s[f, h, ch*128+c']
    w1t = W1s.transpose(2, 0, 1).reshape(C, NF * HID)
    w1h = np.concatenate([w1t[0:128], w1t[128:256]],
                         axis=1).astype(np.float16)
    return np.ascontiguousarray(cw), np.ascontiguousarray(w1h)


def _build():
    if "nc" in _CACHE:
        return _CACHE["nc"]

    nc = bacc.Bacc("TRN2", target_bir_lowering=False, debug=False,
                   num_devices=NCORES)

    # pair-major layout: both samples of a pair are contiguous per
    # channel row, so every input tile is ONE 6656B descriptor/partition
    x = nc.dram_tensor("x", [NPAIR, C, 2, T, VD], F16,
                       kind="ExternalInput").ap()
    # torso-only copy of the LAST pair, streamed first: its gates are
    # computed ~20 us before its bulk data lands, shrinking the tail
    xtor = nc.dram_tensor("xtor", [C, 2, T, 6], F16,
                          kind="ExternalInput").ap()
    cwd = nc.dram_tensor("cw", [128, CW_COLS], F32, kind="ExternalInput").ap()
    w1d = nc.dram_tensor("w1h", [128, 2 * NF * HID], F16,
                         kind="ExternalInput").ap()
    out = nc.dram_tensor("out", [NLOC, C, T, VD], F16,
                         kind="ExternalOutput").ap()

    XY = mybir.AxisListType.XY
    ADD = mybir.AluOpType.add
    MULT = mybir.AluOpType.mult
    MAX = mybir.AluOpType.max

    with tile.TileContext(nc) as tc, ExitStack() as ctx:
        cpool = ctx.enter_context(tc.tile_pool(name="const", bufs=1))
        xpool2 = ctx.enter_context(tc.tile_pool(name="x2", bufs=8))
        opool = ctx.enter_context(tc.tile_pool(name="o", bufs=16))
        spool = ctx.enter_context(tc.tile_pool(name="small", bufs=24))
        pyp = ctx.enter_context(tc.tile_pool(name="py", bufs=2, space="PSUM"))
        php = ctx.enter_context(tc.tile_pool(name="ph", bufs=2, space="PSUM"))
        pgp = ctx.enter_context(tc.tile_pool(name="pg", bufs=4, space="PSUM"))

        # ---- preload ACT tables while the DMA streams ramp up ----------
        dummy = cpool.tile([1, 2], F32, tag="dummy")
        nc.vector.memset(dummy[:], 0.0)
        nc.scalar.activation(dummy[:, 0:1], dummy[:, 0:1],
                             mybir.ActivationFunctionType.Copy, scale=1.0)
        nc.scalar.activation(dummy[:, 1:2], dummy[:, 1:2],
                             mybir.ActivationFunctionType.Sigmoid)

        # ---- replicated constants FIRST on the sync queue (they gate
        # the whole per-pair pipeline, so they must land before the bulk
        # x stream monopolizes the DMA engines) ---------------------------
        cw = cpool.tile([128, CW_COLS], F32, tag="cw")
        nc.sync.dma_start(out=cw[:], in_=cwd)
        w1h = cpool.tile([128, 2 * NF * HID], F16, tag="w1h")
        nc.sync.dma_start(out=w1h[:], in_=w1d)
        w1t = [w1h[:, 0:80], w1h[:, 80:160]]
        w2k = [cw[0:K2, 0:128], cw[0:K2, 128:256]]
        bdmask26 = cw[0:K2, 256:282]
        negb1 = cw[0:NF * HID, 282:283]
        b1x2 = cw[0:NF * HID, 283:284]

        # hs base: rows 96:101 fixed at 1.0 (bias identity), rows 0:80
        # written per sample (column n).
        hsb = cpool.tile([K2, NLOC], F32, tag="hsb")
        nc.vector.memset(hsb[96:K2, :], 1.0)

        # ---- pipeline units: sample pairs ------------------------------
        UNITS = [(2 * p, 2) for p in range(NPAIR)]

        # last pair's torso block, ahead of the bulk stream
        tor_tiles = []
        for ch in range(NCH):
            tt_ = cpool.tile([128, 2, T, 6], F16, tag=f"xtor{ch}")
            nc.sync.dma_start(out=tt_[:], in_=xtor[ch * 128:(ch + 1) * 128])
            tor_tiles.append(tt_)

        # prefetch the whole x shard (sync queue), one tile per unit+chunk
        xtu = []
        for u, (n0, S) in enumerate(UNITS):
            per = []
            for ch in range(NCH):
                xt = xpool2.tile([128, S, T, VD], F16, tag=f"xt{S}",
                                 name=f"xt_{u}_{ch}")
                nc.sync.dma_start(out=xt[:],
                                  in_=x[u, ch * 128:(ch + 1) * 128])
                per.append(xt)
            xtu.append(per)

        state: dict = {}

        def gates_pre(u, src=None):
            """Pools + y matmuls for unit u (independent of older units)."""
            n0, S = UNITS[u]
            if src is None:
                src = xtu[u]
            pv2 = [spool.tile([128, S], F16, tag=f"pv2_{ch}",
                              name=f"pv2_{u}_{ch}") for ch in range(NCH)]
            rs = spool.tile([NF * HID, S], F32, tag="rs", name=f"rs_{u}")
            for ch in range(NCH):
                nc.vector.reduce_max(out=pv2[ch][:],
                                     in_=src[ch][:, :, :, 0:5], axis=XY)
            for i in range(S):
                y = pyp.tile([NF * HID, POOLSZ], F32, tag="y")
                for ch in range(NCH):
                    nc.tensor.matmul(y[:], w1t[ch],
                                     src[ch][:, i, :, 0:5],
                                     start=(ch == 0), stop=(ch == NCH - 1))
                # reduce y's 320 cols on ACT: Copy w/ accum folds the 1/320
                ytr = spool.tile([NF * HID, POOLSZ], F16, tag="ytr")
                nc.scalar.activation(ytr[:], y[:],
                                     mybir.ActivationFunctionType.Copy,
                                     scale=1.0 / POOLSZ,
                                     accum_out=rs[:, i:i + 1])
            phm = php.tile([NF * HID, S], F32, tag="phm")
            for ch in range(NCH):
                nc.tensor.matmul(phm[:], w1t[ch], pv2[ch][:],
                                 start=(ch == 0), stop=(ch == NCH - 1))
            state[u] = (rs, phm)

        def gates_post(u):
            """relu/hs smalls + layer 2 + sigmoid for unit u."""
            n0, S = UNITS[u]
            rs, phm = state.pop(u)
            # hs = relu(avg/320 + b1) + relu(max + b1) via max(z,-b1) + b1
            t1 = spool.tile([NF * HID, S], F32, tag="t1", name=f"t1_{u}")
            t2 = spool.tile([NF * HID, S], F32, tag="t2", name=f"t2_{u}")
            nc.vector.tensor_scalar_max(t1[:], rs[:], negb1)
            nc.vector.tensor_scalar_max(t2[:], phm[:], negb1)
            nc.vector.scalar_tensor_tensor(hsb[0:NF * HID, n0:n0 + S],
                                           t1[:], b1x2, t2[:],
                                           op0=ADD, op1=ADD)
            # block-diag layer-2 operand, pre-expanded to the 26 device
            # columns so layer 2 emits per-COLUMN gate args
            bdk = spool.tile([K2, S * VD], F32, tag="bdk", name=f"bdk_{u}")
            nc.vector.tensor_mul(
                bdk[:].rearrange("p (i v) -> p i v", v=VD),
                bdmask26.unsqueeze(1).broadcast_to([K2, S, VD]),
                hsb[:, n0:n0 + S].unsqueeze(2).broadcast_to([K2, S, VD]))
            gs = []
            for ch in range(NCH):
                pg = pgp.tile([128, S * VD], F32, tag="pg")
                nc.tensor.matmul(pg[:], w2k[ch], bdk[:],
                                 start=True, stop=True)
                gates = spool.tile([128, S * VD], F16, tag="gates",
                                   name=f"gates_{u}_{ch}")
                nc.scalar.activation(gates[:], pg[:],
                                     mybir.ActivationFunctionType.Sigmoid)
                # fp32 copy of the group-4 gate (ACT scale must be fp32)
                g4 = spool.tile([128, S], F32, tag="g4", name=f"g4_{u}_{ch}")
                nc.scalar.activation(
                    g4[:], pg[:].rearrange("p (i v) -> p i v", v=VD)[:, :, 22],
                    mybir.ActivationFunctionType.Sigmoid)
                gs.append((gates, g4))
            state[u] = gs

        def mult_store(u):
            """Gated multiplies + output stores for unit u."""
            n0, S = UNITS[u]
            gs = state.pop(u)
            for ch in range(NCH):
                gates, g4 = gs[ch]
                for i in range(S):
                    n = n0 + i
                    xt = xtu[u][ch][:, i]
                    ot = opool.tile([128, T, VD], F16, tag="ot")
                    grow = gates[:, i * VD:i * VD + TT_COLS] \
                        .unsqueeze(1).broadcast_to([128, T, TT_COLS])
                    nc.vector.tensor_mul(ot[:, :, 0:TT_COLS],
                                         xt[:, :, 0:TT_COLS], grow)
                    g, d0, ln = ACT_RUN
                    nc.scalar.activation(
                        ot[:, :, d0:d0 + ln], xt[:, :, d0:d0 + ln],
                        mybir.ActivationFunctionType.Copy,
                        scale=g4[:, i:i + 1])
                    nc.gpsimd.dma_start(out=out[n, ch * 128:(ch + 1) * 128],
                                        in_=ot[:])

        # natural order: engines are in-order, and the stream is
        # data-paced, so instructions must be emitted in data-arrival
        # order per engine (lookahead causes head-of-line blocking).
        # The LAST pair's gates are computed first from the early torso
        # copy; only its multiplies wait for its bulk tiles.
        LAST = len(UNITS) - 1
        gates_pre(LAST, src=tor_tiles)
        gates_post(LAST)
        for u in range(LAST):
            gates_pre(u)
            gates_post(u)
            mult_store(u)
        mult_store(LAST)

    nc.compile()
    _CACHE["nc"] = nc
    return nc


def _prep(inputs: dict):
    x = np.asarray(inputs["x"])
    # pair-major device layout [pairs, C, 2, T, VD], group-ordered cols
    xdev = np.zeros((N // 2, C, 2, T, VD), dtype=np.float16)
    cols = [j for j in range(VD) if j != PAD_COL]
    xp = x.reshape(N // 2, 2, C, T, V).transpose(0, 2, 1, 3, 4)
    xdev[..., cols] = xp[..., [DEV_SRC[j] for j in cols]].astype(np.float16)
    cw, w1h = _pack_consts(
        np.asarray(inputs["W1s"], dtype=np.float32),
        np.asarray(inputs["b1s"], dtype=np.float32),
        np.asarray(inputs["W2s"], dtype=np.float32),
        np.asarray(inputs["b2s"], dtype=np.float32))
    return [{"x": xdev[i * NPAIR:(i + 1) * NPAIR],
             "xtor": np.ascontiguousarray(
                 xdev[(i + 1) * NPAIR - 1, :, :, :, 0:6]),
             "cw": cw, "w1h": w1h}
            for i in range(NCORES)]


def _post(out_dev):
    # device order is already the reference output order; drop the pad col
    return out_dev[..., DEV_SEL].astype(np.float32)


def run(inputs: dict, trace: bool = False, **kw):
    nc = _build()
    in_maps = _prep(inputs)
    res = run_bass_kernel_spmd(nc, in_maps, list(range(NCORES)),
                               trace=trace, **kw)
    full = np.concatenate([res.results[i]["out"] for i in range(NCORES)],
                          axis=0)
    return _post(full), res


def _runner():
    """Build (once) a cached jitted SPMD callable: full inputs -> full out."""
    if "call" in _CACHE:
        return _CACHE["call"]
    import jax
    from jax.sharding import Mesh, PartitionSpec
    from jax.experimental.shard_map import shard_map
    from concourse import bass2jax, mybir as mb

    nc = _build()
    bass2jax.install_neuronx_cc_hook()

    in_names, out_names, out_avals, zero_outs = [], [], [], []
    for alloc in nc.m.functions[0].allocations:
        if not isinstance(alloc, mb.MemoryLocationSet):
            continue
        name = alloc.memorylocations[0].name
        if alloc.kind == "ExternalInput":
            in_names.append(name)
        elif alloc.kind == "ExternalOutput":
            shape = tuple(alloc.tensor_shape)
            dtype = mb.dt.np(alloc.dtype)
            out_names.append(name)
            out_avals.append(jax.core.ShapedArray(shape, dtype))
            zero_outs.append(np.zeros(shape, dtype))
    n_params = len(in_names)

    def _body(*args):
        return tuple(bass2jax._bass_exec_p.bind(
            *args,
            out_avals=tuple(out_avals),
            in_names=tuple(in_names + out_names),
            out_names=tuple(out_names),
            lowering_input_output_aliases=(),
            sim_require_finite=True,
            sim_require_nnan=True,
            nc=nc,
        ))

    devices = jax.devices()[:NCORES]
    mesh = Mesh(np.asarray(devices), ("core",))
    nio = n_params + len(out_names)
    sharded = jax.jit(
        shard_map(_body, mesh=mesh,
                  in_specs=(PartitionSpec("core"),) * nio,
                  out_specs=(PartitionSpec("core"),) * len(out_names),
                  check_rep=False),
        donate_argnums=tuple(range(n_params, nio)),
        keep_unused=True,
    )
    cz = [np.zeros((NCORES * z.shape[0], *z.shape[1:]), z.dtype)
          for z in zero_outs]

    def call(in_maps):
        concat_in = [np.concatenate([m[name] for m in in_maps], axis=0)
                     for name in in_names]
        outs = sharded(*concat_in, *[z.copy() for z in cz])
        return np.asarray(outs[out_names.index("out")])

    _CACHE["call"] = call
    return call


def kernel(**inputs) -> np.ndarray:
    in_maps = _prep(inputs)
    try:
        call = _runner()
        return _post(call(in_maps))
    except Exception:
        full, _ = run(inputs)
        return full
